# revision 39
# baseline (speedup 1.0000x reference)
"""BinaryLinear on 8 trn2 NeuronCores: y = x @ sign(W)^T + bias.

x: (8192, 4096) f32, W: (4096, 4096) f32, bias: (4096,) f32 -> y: (8192, 4096) f32.

Strategy
--------
Data-parallel: shard x rows 8 x 1024 across cores; every core holds the full
binarized weight. No collectives; host concatenates the output shards.

Per-core Bass kernel (M=1024, K=4096, O=4096):
  - dtype float32r for both matmul operands: measured 227 ns per
    (128k x 128o x 512m) matmul on trn2 (within 5% of bf16 rate) with
    ~1e-4 relative error (tf32-like operand rounding) -- far more accurate
    than bf16's ~2e-3.
  - stationary (lhsT) = sign(W)^T tile [128k, 128o]; moving (rhs) = x^T block
    [128k, 512m]; PSUM out = y^T tile [128o, 512m] fp32. k-innermost loop: a
    full 32-step accumulation group per PSUM bank, banks rotate, so the PE
    never stalls on drains.
  - sign panels ship as bf16 (+-1/0 are exact) and are expanded to f32r on
    the vector engine, halving their DMA cost.
  - Ramp: the 16.8 MB x load is DMA-bound (~42 us at ~400 GB/s). The first
    six (o-tile, m-block) accumulation groups interleave k-outermost in
    program order, so the in-order PE issues 6 matmuls (~1.4 us) per arriving
    x chunk (~1.3 us) instead of idling behind the full load.
  - Output is produced transposed (y^T), which puts the bias on the PSUM
    partition axis: one per-partition tensor_scalar_add fuses bias + PSUM
    eviction. Host transposes shards back.
  - All operands are host-packed so every DMA is a fully contiguous
    partition-major block.
"""

import numpy as np
import ml_dtypes

import concourse.bass as bass  # noqa: F401  (registers engine types)
import concourse.tile as tile
from concourse import bacc, mybir
from concourse.bass_utils import run_bass_kernel_spmd

NCORES = 8
M_FULL, K, O = 8192, 4096, 4096
M = M_FULL // NCORES          # 1024 rows of x per core
P = 128                       # partition width
KO = K // P                   # 32 k-tiles
OT = O // P                   # 32 o-tiles
NM = 512                      # moving free dim per matmul
MB = M // NM                  # 2 m-blocks
RAMP_OT = 4                   # o-tiles interleaved k-outer during the x load

_F32R = mybir.dt.float32r
_F32 = mybir.dt.float32
_BF16 = mybir.dt.bfloat16

_COMPILED = None


def _build():
    nc = bacc.Bacc("TRN2", target_bir_lowering=False, debug=False)
    xt_ap = nc.dram_tensor("xt", [P, KO, M], _F32R, kind="ExternalInput").ap()
    st_ap = nc.dram_tensor("st", [OT, P, KO, P], _BF16, kind="ExternalInput").ap()
    b_ap = nc.dram_tensor("biasc", [P, OT], _F32, kind="ExternalInput").ap()
    yt_ap = nc.dram_tensor("yt", [O, M], _F32, kind="ExternalOutput").ap()
    yt_r = yt_ap.rearrange("(ot p) m -> ot p m", p=P)

    from contextlib import ExitStack

    with tile.TileContext(nc) as tc:
        with ExitStack() as ctx:
            xpool = ctx.enter_context(tc.tile_pool(name="x", bufs=KO))
            srpool = ctx.enter_context(tc.tile_pool(name="sr", bufs=4))
            sepool = ctx.enter_context(tc.tile_pool(name="se", bufs=4))
            bpool = ctx.enter_context(tc.tile_pool(name="b", bufs=1))
            ypool = ctx.enter_context(tc.tile_pool(name="y", bufs=3))
            psum = ctx.enter_context(tc.tile_pool(name="ps", bufs=8, space="PSUM"))

            b_sb = bpool.tile([P, OT], _F32)
            nc.sync.dma_start(b_sb[:], b_ap[:])

            # Prewarm the PE so HAM un-throttles (1.2 -> 2.4 GHz) before the
            # real matmuls: ~5 us of dummy work on a scratch tile, discarded.
            scratch = bpool.tile([P, 256], _F32)
            nc.vector.memset(scratch[:], 1.0)
            warm_ps = psum.tile([P, 256], _F32, name="ps_warm", tag="ps")
            for _ in range(12):
                nc.tensor.matmul(
                    warm_ps[:], scratch[:, :P], scratch[:], start=True, stop=True
                )

            EXP_PC = 8  # ko per expansion piece (finer deps -> earlier matmuls)

            def load_pieces(ot):
                """DMA one bf16 piece + expand to f32r; raw staging is
                piece-granular so only 8 KB/partition of staging is live.
                Sign DMAs ride the GpSimd queue so their triggers never
                serialize ahead of the x chunks on the Sync queue."""
                exp = sepool.tile([P, KO, P], _F32R, name=f"sexp{ot}", tag="sexp")
                for pc in range(0, KO, EXP_PC):
                    raw = srpool.tile(
                        [P, EXP_PC, P], _BF16, name=f"sraw{ot}_{pc}", tag="sraw"
                    )
                    nc.gpsimd.dma_start(raw[:], st_ap[ot][:, pc:pc + EXP_PC, :])
                    nc.vector.tensor_copy(exp[:, pc:pc + EXP_PC, :], raw[:])
                return exp

            # Whole x^T shard resident in SBUF (16.8 MB), one tile per k-tile
            # so matmuls only depend on the chunk they read. The first chunks
            # split into m-block halves: DMA spin-up delivers them with finer
            # granularity, so the mb-major ramp groups start sooner.
            X_SPLIT = 16
            x_tiles = []
            for ko in range(KO):
                xt = xpool.tile([P, M], _F32R, name=f"x{ko}", tag="x")
                if ko < X_SPLIT:
                    for h in range(MB):
                        nc.sync.dma_start(
                            xt[:, h * NM:(h + 1) * NM],
                            xt_ap[:, ko, h * NM:(h + 1) * NM],
                        )
                else:
                    nc.sync.dma_start(xt[:], xt_ap[:, ko, :])
                x_tiles.append(xt)

            # Ramp sign panels (bf16, small) stream on the GpSimd queue in
            # parallel with the x load; the bf16->f32r expansion pieces are
            # interleaved across panels so every panel's first k-tiles are
            # ready as soon as possible.
            s_tiles = {
                ot: sepool.tile([P, KO, P], _F32R, name=f"sexp{ot}", tag="sexp")
                for ot in range(RAMP_OT)
            }
            for pc in range(0, KO, EXP_PC):
                for ot in range(RAMP_OT):
                    raw = srpool.tile(
                        [P, EXP_PC, P], _BF16, name=f"sraw{ot}_{pc}", tag="sraw"
                    )
                    nc.gpsimd.dma_start(raw[:], st_ap[ot][:, pc:pc + EXP_PC, :])
                    nc.vector.tensor_copy(
                        s_tiles[ot][:, pc:pc + EXP_PC, :], raw[:]
                    )

            def drain(ps, ot, mb):
                y_sb = ypool.tile([P, NM], _F32, name=f"y{ot}_{mb}", tag="y")
                nc.vector.tensor_scalar_add(y_sb[:], ps[:], b_sb[:, ot:ot + 1])
                nc.sync.dma_start(yt_r[ot][:, mb * NM:(mb + 1) * NM], y_sb[:])

            # Ramp: k-outer over the first RAMP_OT panels' groups, so the PE
            # issues work for x chunk k as soon as that chunk's DMA lands
            # instead of stalling in-order behind the full x load. Half the
            # groups lag by CATCH k-rows: their matmuls read chunks that
            # landed ~5 us earlier, giving the in-order PE a guaranteed-ready
            # backlog that absorbs chunk-arrival jitter.
            groups = [(ot, mb) for mb in range(MB) for ot in range(RAMP_OT)]
            lead, reserve = groups[:4], groups[4:]
            CATCH = 4
            ramp_ps = {
                g: psum.tile([P, NM], _F32, name=f"ps_r{g[0]}_{g[1]}", tag="ps")
                for g in groups
            }

            def ramp_mm(g, k):
                ot, mb = g
                nc.tensor.matmul(
                    ramp_ps[g][:],
                    s_tiles[ot][:, k, :],
                    x_tiles[k][:, mb * NM:(mb + 1) * NM],
                    start=(k == 0),
                    stop=(k == KO - 1),
                )

            for k in range(KO + CATCH):
                if k < KO:
                    for g in lead:
                        ramp_mm(g, k)
                if k >= CATCH:
                    for g in reserve:
                        ramp_mm(g, k - CATCH)
            # Prefetch the first steady panel before the ramp drains so its
            # DVE expansion isn't queued behind them.
            s_next = load_pieces(RAMP_OT)
            for (ot, mb) in groups:
                drain(ramp_ps[(ot, mb)], ot, mb)

            # Steady state: k-inner accumulation, one PSUM bank per group.
            for ot in range(RAMP_OT, OT):
                s_sb = s_next if ot == RAMP_OT else load_pieces(ot)
                for mb in range(MB):
                    ps = psum.tile([P, NM], _F32)
                    for k in range(KO):
                        nc.tensor.matmul(
                            ps[:],
                            s_sb[:, k, :],
                            x_tiles[k][:, mb * NM:(mb + 1) * NM],
                            start=(k == 0),
                            stop=(k == KO - 1),
                        )
                    drain(ps, ot, mb)

    nc.compile()
    return nc


def _get_compiled():
    global _COMPILED
    if _COMPILED is None:
        _COMPILED = _build()
    return _COMPILED


def _pack_inputs(x, weight, bias):
    x = np.ascontiguousarray(x, dtype=np.float32)
    s = np.sign(weight).astype(np.float32)
    # st[ot, ki, ko, o] = s[ot*128 + o, ko*128 + ki]; +-1/0 are exact in bf16.
    st = np.ascontiguousarray(
        s.reshape(OT, P, KO, P).transpose(0, 3, 2, 1).astype(ml_dtypes.bfloat16)
    )
    biasc = np.ascontiguousarray(
        np.asarray(bias, dtype=np.float32).reshape(OT, P).T
    )
    in_maps = []
    for c in range(NCORES):
        xs = x[c * M:(c + 1) * M]                     # (M, K)
        # xt[ki, ko, m] = xs[m, ko*128 + ki]
        xt = np.ascontiguousarray(xs.reshape(M, KO, P).transpose(2, 1, 0))
        in_maps.append({"xt": xt, "st": st, "biasc": biasc})
    return in_maps


def _run(x, weight, bias, trace=False):
    nc = _get_compiled()
    in_maps = _pack_inputs(x, weight, bias)
    res = run_bass_kernel_spmd(nc, in_maps, list(range(NCORES)), trace=trace)
    y = np.empty((M_FULL, O), dtype=np.float32)
    for c in range(NCORES):
        y[c * M:(c + 1) * M] = res.results[c]["yt"].T
    return y, res


def kernel(x, weight, bias):
    y, _ = _run(x, weight, bias, trace=False)
    return y


# revision 40
# speedup vs baseline: 1.0066x; 1.0066x over previous
"""BinaryLinear on 8 trn2 NeuronCores: y = x @ sign(W)^T + bias.

x: (8192, 4096) f32, W: (4096, 4096) f32, bias: (4096,) f32 -> y: (8192, 4096) f32.

Strategy
--------
Data-parallel: shard x rows 8 x 1024 across cores; every core holds the full
binarized weight. No collectives; host concatenates the output shards.

Per-core Bass kernel (M=1024, K=4096, O=4096):
  - dtype float32r for both matmul operands: measured 227 ns per
    (128k x 128o x 512m) matmul on trn2 (within 5% of bf16 rate) with
    ~1e-4 relative error (tf32-like operand rounding) -- far more accurate
    than bf16's ~2e-3.
  - stationary (lhsT) = sign(W)^T tile [128k, 128o]; moving (rhs) = x^T block
    [128k, 512m]; PSUM out = y^T tile [128o, 512m] fp32. k-innermost loop: a
    full 32-step accumulation group per PSUM bank, banks rotate, so the PE
    never stalls on drains.
  - sign panels ship as bf16 (+-1/0 are exact) and are expanded to f32r on
    the vector engine, halving their DMA cost.
  - Ramp: the 16.8 MB x load is DMA-bound (~42 us at ~400 GB/s). The first
    six (o-tile, m-block) accumulation groups interleave k-outermost in
    program order, so the in-order PE issues 6 matmuls (~1.4 us) per arriving
    x chunk (~1.3 us) instead of idling behind the full load.
  - Output is produced transposed (y^T), which puts the bias on the PSUM
    partition axis: one per-partition tensor_scalar_add fuses bias + PSUM
    eviction. Host transposes shards back.
  - All operands are host-packed so every DMA is a fully contiguous
    partition-major block.
"""

import numpy as np
import ml_dtypes

import concourse.bass as bass  # noqa: F401  (registers engine types)
import concourse.tile as tile
from concourse import bacc, mybir
from concourse.bass_utils import run_bass_kernel_spmd

NCORES = 8
M_FULL, K, O = 8192, 4096, 4096
M = M_FULL // NCORES          # 1024 rows of x per core
P = 128                       # partition width
KO = K // P                   # 32 k-tiles
OT = O // P                   # 32 o-tiles
NM = 512                      # moving free dim per matmul
MB = M // NM                  # 2 m-blocks
RAMP_OT = 4                   # o-tiles interleaved k-outer during the x load

_F32R = mybir.dt.float32r
_F32 = mybir.dt.float32
_BF16 = mybir.dt.bfloat16

_COMPILED = None


def _build():
    nc = bacc.Bacc("TRN2", target_bir_lowering=False, debug=False)
    xt_ap = nc.dram_tensor("xt", [P, KO, M], _F32R, kind="ExternalInput").ap()
    st_ap = nc.dram_tensor("st", [OT, P, KO, P], _BF16, kind="ExternalInput").ap()
    b_ap = nc.dram_tensor("biasc", [P, OT], _F32, kind="ExternalInput").ap()
    yt_ap = nc.dram_tensor("yt", [O, M], _F32, kind="ExternalOutput").ap()
    yt_r = yt_ap.rearrange("(ot p) m -> ot p m", p=P)

    from contextlib import ExitStack

    with tile.TileContext(nc) as tc:
        with ExitStack() as ctx:
            xpool = ctx.enter_context(tc.tile_pool(name="x", bufs=KO))
            srpool = ctx.enter_context(tc.tile_pool(name="sr", bufs=4))
            sepool = ctx.enter_context(tc.tile_pool(name="se", bufs=4))
            bpool = ctx.enter_context(tc.tile_pool(name="b", bufs=1))
            ypool = ctx.enter_context(tc.tile_pool(name="y", bufs=3))
            psum = ctx.enter_context(tc.tile_pool(name="ps", bufs=8, space="PSUM"))

            b_sb = bpool.tile([P, OT], _F32)
            nc.sync.dma_start(b_sb[:], b_ap[:])

            # Prewarm the PE so HAM un-throttles (1.2 -> 2.4 GHz) before the
            # real matmuls: ~5 us of dummy work on a scratch tile, discarded.
            scratch = bpool.tile([P, 256], _F32)
            nc.vector.memset(scratch[:], 1.0)
            warm_ps = psum.tile([P, 256], _F32, name="ps_warm", tag="ps")
            for _ in range(12):
                nc.tensor.matmul(
                    warm_ps[:], scratch[:, :P], scratch[:], start=True, stop=True
                )

            EXP_PC = 8  # ko per expansion piece (finer deps -> earlier matmuls)

            def load_pieces(ot):
                """DMA one bf16 piece + expand to f32r; raw staging is
                piece-granular so only 8 KB/partition of staging is live.
                Sign DMAs ride the GpSimd queue so their triggers never
                serialize ahead of the x chunks on the Sync queue."""
                exp = sepool.tile([P, KO, P], _F32R, name=f"sexp{ot}", tag="sexp")
                for pc in range(0, KO, EXP_PC):
                    raw = srpool.tile(
                        [P, EXP_PC, P], _BF16, name=f"sraw{ot}_{pc}", tag="sraw"
                    )
                    nc.gpsimd.dma_start(raw[:], st_ap[ot][:, pc:pc + EXP_PC, :])
                    nc.vector.tensor_copy(exp[:, pc:pc + EXP_PC, :], raw[:])
                return exp

            # Whole x^T shard resident in SBUF (16.8 MB), one tile per k-tile
            # so matmuls only depend on the chunk they read. The first chunks
            # split into m-block halves: DMA spin-up delivers them with finer
            # granularity, so the mb-major ramp groups start sooner.
            X_SPLIT = 16
            x_tiles = []
            for ko in range(KO):
                xt = xpool.tile([P, M], _F32R, name=f"x{ko}", tag="x")
                if ko < X_SPLIT:
                    for h in range(MB):
                        nc.sync.dma_start(
                            xt[:, h * NM:(h + 1) * NM],
                            xt_ap[:, ko, h * NM:(h + 1) * NM],
                        )
                else:
                    nc.sync.dma_start(xt[:], xt_ap[:, ko, :])
                x_tiles.append(xt)

            # Ramp sign panels (bf16, small) stream on the GpSimd queue in
            # parallel with the x load; the bf16->f32r expansion pieces are
            # interleaved across panels so every panel's first k-tiles are
            # ready as soon as possible.
            s_tiles = {
                ot: sepool.tile([P, KO, P], _F32R, name=f"sexp{ot}", tag="sexp")
                for ot in range(RAMP_OT)
            }
            for pc in range(0, KO, EXP_PC):
                for ot in range(RAMP_OT):
                    raw = srpool.tile(
                        [P, EXP_PC, P], _BF16, name=f"sraw{ot}_{pc}", tag="sraw"
                    )
                    nc.gpsimd.dma_start(raw[:], st_ap[ot][:, pc:pc + EXP_PC, :])
                    nc.vector.tensor_copy(
                        s_tiles[ot][:, pc:pc + EXP_PC, :], raw[:]
                    )

            def drain(ps, ot, mb):
                y_sb = ypool.tile([P, NM], _F32, name=f"y{ot}_{mb}", tag="y")
                nc.vector.tensor_scalar_add(y_sb[:], ps[:], b_sb[:, ot:ot + 1])
                nc.sync.dma_start(yt_r[ot][:, mb * NM:(mb + 1) * NM], y_sb[:])

            # Ramp: k-outer over the first RAMP_OT panels' groups, so the PE
            # issues work for x chunk k as soon as that chunk's DMA lands
            # instead of stalling in-order behind the full x load.
            groups = [(ot, mb) for mb in range(MB) for ot in range(RAMP_OT)]
            ramp_ps = {
                g: psum.tile([P, NM], _F32, name=f"ps_r{g[0]}_{g[1]}", tag="ps")
                for g in groups
            }
            for k in range(KO):
                for (ot, mb) in groups:
                    nc.tensor.matmul(
                        ramp_ps[(ot, mb)][:],
                        s_tiles[ot][:, k, :],
                        x_tiles[k][:, mb * NM:(mb + 1) * NM],
                        start=(k == 0),
                        stop=(k == KO - 1),
                    )
            # Prefetch the first steady panel before the ramp drains so its
            # DVE expansion isn't queued behind them.
            s_next = load_pieces(RAMP_OT)
            for (ot, mb) in groups:
                drain(ramp_ps[(ot, mb)], ot, mb)

            # Steady state: k-inner accumulation, one PSUM bank per group.
            for ot in range(RAMP_OT, OT):
                s_sb = s_next if ot == RAMP_OT else load_pieces(ot)
                for mb in range(MB):
                    ps = psum.tile([P, NM], _F32)
                    for k in range(KO):
                        nc.tensor.matmul(
                            ps[:],
                            s_sb[:, k, :],
                            x_tiles[k][:, mb * NM:(mb + 1) * NM],
                            start=(k == 0),
                            stop=(k == KO - 1),
                        )
                    drain(ps, ot, mb)

    nc.compile()
    return nc


def _get_compiled():
    global _COMPILED
    if _COMPILED is None:
        _COMPILED = _build()
    return _COMPILED


def _pack_inputs(x, weight, bias):
    x = np.ascontiguousarray(x, dtype=np.float32)
    s = np.sign(weight).astype(np.float32)
    # st[ot, ki, ko, o] = s[ot*128 + o, ko*128 + ki]; +-1/0 are exact in bf16.
    st = np.ascontiguousarray(
        s.reshape(OT, P, KO, P).transpose(0, 3, 2, 1).astype(ml_dtypes.bfloat16)
    )
    biasc = np.ascontiguousarray(
        np.asarray(bias, dtype=np.float32).reshape(OT, P).T
    )
    in_maps = []
    for c in range(NCORES):
        xs = x[c * M:(c + 1) * M]                     # (M, K)
        # xt[ki, ko, m] = xs[m, ko*128 + ki]
        xt = np.ascontiguousarray(xs.reshape(M, KO, P).transpose(2, 1, 0))
        in_maps.append({"xt": xt, "st": st, "biasc": biasc})
    return in_maps


def _run(x, weight, bias, trace=False):
    nc = _get_compiled()
    in_maps = _pack_inputs(x, weight, bias)
    res = run_bass_kernel_spmd(nc, in_maps, list(range(NCORES)), trace=trace)
    y = np.empty((M_FULL, O), dtype=np.float32)
    for c in range(NCORES):
        y[c * M:(c + 1) * M] = res.results[c]["yt"].T
    return y, res


def kernel(x, weight, bias):
    y, _ = _run(x, weight, bias, trace=False)
    return y
